# revision 25
# baseline (speedup 1.0000x reference)
"""Trainium2 Bass kernel for a relational GCN layer — dense count-matmul.

Math (reference):
  S = feat[src]; msgs[e] = edge_nn(S[e], W_rel[rel[e]]) (tied 2-layer relu MLP)
  agg = segment_sum(msgs, dst, N); hn = LSTM-step(agg); out = MLP(hn)

Messages depend only on (rel, src): H[r, s] = edge_nn(feat[s], W_rel[r]) has
NUM_REL*N = 20k rows << E = 320k.  agg[n] = sum_{r,s} C[n, (r,s)] * H[(r,s)]
where C is the per-node edge-count matrix (dense fp8, exact small ints),
built host-side and streamed from HBM.

Distribution: edges sharded by DESTINATION range (core c owns dst in
[1250c, 1250(c+1))); every core computes the full H table locally (an
AllGather-based exchange was tried and measured slower: the collective's
mesh algorithm plus launch-skew rendezvous costs more PE idle than the
duplicated 69us of phase A).

Schedule: the aggregation runs dst-chunk-major (C streamed chunk-major,
each byte once), so chunk ci's accumulators close after its k-sweep and
the per-node LSTM+MLP for its dst blocks overlaps the remaining chunks'
matmuls, hiding most of the epilogue.  PSUM: 2 banks phase A + 4 banks
agg (2 feat-halves x double-buffer) + 2 banks phase C.
"""

import numpy as np
import ml_dtypes

import concourse.bacc as bacc
import concourse.bass as bass
import concourse.mybir as mybir
import concourse.tile as tile
from concourse import bass_utils
from concourse.masks import make_identity

# ---- problem constants (hardcoded per spec) ----
N_NODES = 10000
N_EDGES = 320000
D = 256
D_OUT = 256
NUM_REL = 2
NCORES = 8
NPC = N_NODES // NCORES          # 1250 dst nodes per core
NBLK = 10                        # dst blocks per core (last has 98 rows)
NDST = NPC                       # exact dst cols per core (no padding)
NPAD = 10240                     # src nodes padded to 20 chunks of 512
NROWS = NUM_REL * NPAD           # 20480 H-table rows
NKT = NROWS // 128               # 160 k-tiles
GK = 8                           # k-tiles per C DMA tile (4KB lines)
NJ = NKT // GK                   # 40 C DMA tiles per chunk
CHUNKS = [(0, 512), (512, 512), (1024, 226)]    # dst col chunks (sum 1250)
A_CHUNKS = [(c * 512, 512) for c in range(NPAD // 512)]  # src chunks

f32 = mybir.dt.float32
f32r = mybir.dt.float32r
bf16 = mybir.dt.bfloat16
fp8 = mybir.dt.float8e4

_np_bf16 = ml_dtypes.bfloat16
_np_fp8 = ml_dtypes.float8_e4m3


# ----------------------------------------------------------------------------
# host-side preprocessing
# ----------------------------------------------------------------------------

def _prep_counts(src, dst, rel):
    """Per-core dense count matrices in fp8, chunk-major, GK k-tiles per
    DMA line.  Returns per-core dicts {"C0","C1","C2"} with shapes
    [NJ, 128, GK, cw]."""
    row = rel.astype(np.int64) * NPAD + src.astype(np.int64)
    core = dst // NPC
    col = (dst % NPC).astype(np.int64)
    counts = np.zeros((NCORES, NROWS, NDST), dtype=np.uint8)
    np.add.at(counts, (core, row, col), 1)
    out = []
    for c in range(NCORES):
        cc = counts[c].reshape(NJ, GK, 128, NDST).transpose(0, 2, 1, 3)
        m = {}
        for ci, (c0, cw) in enumerate(CHUNKS):
            m[f"C{ci}"] = np.ascontiguousarray(
                cc[:, :, :, c0:c0 + cw]).astype(_np_fp8)
        out.append(m)
    return out


def _prep_weights(inputs):
    feat = np.asarray(inputs["feat"], dtype=np.float32)
    W_rel = np.asarray(inputs["W_rel"], dtype=np.float32)
    b_rel = np.asarray(inputs["b_rel"], dtype=np.float32)
    W_ih = np.asarray(inputs["W_ih"], dtype=np.float32)
    b_ih = np.asarray(inputs["b_ih"], dtype=np.float32)
    b_hh = np.asarray(inputs["b_hh"], dtype=np.float32)
    W1 = np.asarray(inputs["W1"], dtype=np.float32)
    W2 = np.asarray(inputs["W2"], dtype=np.float32)
    W3 = np.asarray(inputs["W3"], dtype=np.float32)

    featT = np.zeros((D, NPAD), dtype=np.float32)
    featT[:, :N_NODES] = feat.T
    keep = np.r_[0:256, 512:1024]  # i, g, o gate columns (f unused: c0 = 0)

    # bf16 pack: W_rT (4x[128,256]) | W1T (2x[128,128]) | W2T | W3T [128,256]
    W_rT = np.transpose(W_rel, (0, 2, 1))  # [rel, in, out]
    pack_b = np.concatenate(
        [W_rT[0, 0:128], W_rT[0, 128:256], W_rT[1, 0:128], W_rT[1, 128:256],
         W1.T[0:128], W1.T[128:256], W2.T, W3.T], axis=1)  # [128, 1664]
    # f32 pack: b_r_col 4x[128,1] | b1|b2|b3a|b3b 4x[128,1] |
    #           b_r_rep 2x[128,256] | b_g_rep [128,768]
    b_r_col = b_rel.reshape(NUM_REL * 2, 128, 1)
    b3 = np.asarray(inputs["b3"]).reshape(2, 128, 1)
    pack_f = np.concatenate(
        [b_r_col[0], b_r_col[1], b_r_col[2], b_r_col[3],
         np.asarray(inputs["b1"]).reshape(128, 1),
         np.asarray(inputs["b2"]).reshape(128, 1), b3[0], b3[1],
         np.broadcast_to(b_rel[0][None, :], (128, D)),
         np.broadcast_to(b_rel[1][None, :], (128, D)),
         np.broadcast_to((b_ih + b_hh)[keep][None, :], (128, 768))],
        axis=1).astype(np.float32)  # [128, 1288]

    com = {
        "pack_b": np.ascontiguousarray(pack_b).astype(_np_bf16),
        "pack_f": np.ascontiguousarray(pack_f),
        "W_ihT": np.ascontiguousarray(W_ih.T[:, keep]).astype(np.float32),
        "fts": featT.astype(_np_bf16),
    }
    return com


# ----------------------------------------------------------------------------
# kernel builder
# ----------------------------------------------------------------------------

def _build():
    Relu = mybir.ActivationFunctionType.Relu
    Sig = mybir.ActivationFunctionType.Sigmoid
    Tanh = mybir.ActivationFunctionType.Tanh

    nc = bacc.Bacc("TRN2", target_bir_lowering=False, debug=False)

    fts_d = nc.dram_tensor("fts", [D, NPAD], bf16, kind="ExternalInput")
    pack_b_d = nc.dram_tensor("pack_b", [128, 1664], bf16, kind="ExternalInput")
    pack_f_d = nc.dram_tensor("pack_f", [128, 1288], f32, kind="ExternalInput")
    W_ihT_d = nc.dram_tensor("W_ihT", [D, 768], f32, kind="ExternalInput")
    C_ds = [nc.dram_tensor(f"C{ci}", [NJ, 128, GK, cw], fp8,
                           kind="ExternalInput")
            for ci, (c0, cw) in enumerate(CHUNKS)]
    outT_d = nc.dram_tensor("outT", [D_OUT, NPC], f32, kind="ExternalOutput")

    with tile.TileContext(nc) as tc:
        with (
            tc.tile_pool(name="const", bufs=1) as cp,
            tc.tile_pool(name="work", bufs=3) as wp,
            tc.tile_pool(name="hbig", bufs=1) as hp_pool,
            tc.tile_pool(name="aggpool", bufs=1) as ap_pool,
        ):
            # ---- constants, ordered so phase A's first chunk starts ASAP
            pb = cp.tile([128, 1664], bf16, tag="pb")
            nc.sync.dma_start(pb[:, 0:512], pack_b_d[:, 0:512])
            pf = cp.tile([128, 1288], f32, tag="pf")
            nc.scalar.dma_start(pf[:, 0:8], pack_f_d[:, 0:8])
            ft = {}
            for h in range(2):
                ft[h] = cp.tile([128, NPAD], bf16, tag=f"ft{h}",
                                name=f"ft{h}")
            FT_PIECES = [(0, 512), (512, 1536), (2048, 2048), (4096, 3072),
                         (7168, 3072)]
            for p, (p0, pw) in enumerate(FT_PIECES):
                for h in range(2):
                    eng = [nc.sync, nc.scalar, nc.gpsimd][(p * 2 + h) % 3]
                    eng.dma_start(ft[h][:, p0:p0 + pw],
                                  fts_d[h * 128:(h + 1) * 128, p0:p0 + pw])
            nc.sync.dma_start(pf[:, 8:520], pack_f_d[:, 8:520])
            nc.scalar.dma_start(pb[:, 512:1024], pack_b_d[:, 512:1024])
            nc.gpsimd.dma_start(pb[:, 1024:1664], pack_b_d[:, 1024:1664])
            nc.gpsimd.dma_start(pf[:, 520:1288], pack_f_d[:, 520:1288])
            W_ihT_sb = {}
            for h in range(2):
                t = wp.tile([128, 768], f32, tag=f"wih{h}", bufs=1)
                nc.gpsimd.dma_start(t[:], W_ihT_d[h * 128:(h + 1) * 128, :])
                W_ihT_sb[h] = t
            # pack slices (views)
            W_rT_sb = {(r, h): pb[:, (r * 2 + h) * 256:(r * 2 + h + 1) * 256]
                       for r in range(NUM_REL) for h in range(2)}
            W1T_sb = {h: pb[:, 1024 + h * 128:1024 + (h + 1) * 128]
                      for h in range(2)}
            W2T_sb = pb[:, 1280:1408]
            W3T_sb = pb[:, 1408:1664]
            b_r_col_sb = {(r, h): pf[:, r * 2 + h:r * 2 + h + 1]
                          for r in range(NUM_REL) for h in range(2)}
            b1_col_sb = pf[:, 4:5]
            b2_col_sb = pf[:, 5:6]
            b3_col_sb = {h: pf[:, 6 + h:7 + h] for h in range(2)}
            b_r_rep_sb = {r: pf[:, 8 + r * 256:8 + (r + 1) * 256]
                          for r in range(NUM_REL)}
            b_g_rep_sb = pf[:, 520:1288]

            ident = cp.tile([128, 128], f32, tag="ident")
            make_identity(nc, ident[:])

            # ---- H table, built in SBUF by phase A, consumed by the agg ----
            hbuf = hp_pool.tile([128, NKT, D], bf16, tag="hbuf")

            # ---- phase A: H[r*NPAD + s] = edge_nn(feat[s], W_rel[r]) ----
            psAB = tc.tile_pool(name="psA", bufs=1, space="PSUM")
            psAggB = tc.tile_pool(name="psAgg", bufs=1, space="PSUM")
            psA = psAB.__enter__()
            psAgg = psAggB.__enter__()
            for r in range(NUM_REL):
                for (c0, cw) in A_CHUNKS:
                    z1s = {}
                    for do_h in range(2):
                        z1p = psA.tile([128, 512], f32, tag="z1",
                                       space="PSUM", bufs=2)
                        for di_h in range(2):
                            nc.tensor.matmul(
                                z1p[:],
                                lhsT=W_rT_sb[r, di_h][
                                    :, do_h * 128:(do_h + 1) * 128],
                                rhs=ft[di_h][:, c0:c0 + cw],
                                start=(di_h == 0), stop=(di_h == 1))
                        z = wp.tile([128, 512], bf16, tag=f"z1s{do_h}",
                                    bufs=2)
                        nc.vector.tensor_scalar(
                            z[:], z1p[:], b_r_col_sb[r, do_h], 0.0,
                            op0=mybir.AluOpType.add, op1=mybir.AluOpType.max)
                        z1s[do_h] = z
                    for c4 in range(cw // 128):
                        kt = r * (NKT // 2) + (c0 // 128) + c4
                        hp = psA.tile([128, D], f32, tag="hp",
                                      space="PSUM", bufs=4)
                        sl = slice(c4 * 128, (c4 + 1) * 128)
                        nc.tensor.matmul(hp[:], lhsT=z1s[0][:, sl],
                                         rhs=W_rT_sb[r, 0][:],
                                         start=True, stop=False)
                        nc.tensor.matmul(hp[:], lhsT=z1s[1][:, sl],
                                         rhs=W_rT_sb[r, 1][:],
                                         start=False, stop=True)
                        nc.vector.tensor_add(hp[:], hp[:], b_r_rep_sb[r])
                        nc.vector.tensor_scalar_max(hbuf[:, kt, :], hp[:], 0.0)

            # aggT in SBUF: [feat-half 128][1280], f32 (used as f32r);
            # cols 1250:1280 feed only the unused tail columns of the last
            # phase-C block (never DMA'd out).
            aggT_sb = {}
            for h in range(2):
                aggT_sb[h] = ap_pool.tile([128, 1280], f32r,
                                          tag=f"aggT{h}", name=f"aggT{h}")

            # ---- agg (dst-chunk-major): one chunk's accumulators
            # live at a time (2 PSUM banks); C streamed chunk-major so each
            # byte is read once.
            aggp_all = []
            for ci, (c0, cw) in enumerate(CHUNKS):
                aggp = {}
                for h in range(2):
                    aggp[h] = psAgg.tile([128, 512], f32, tag=f"aggp{h}",
                                         space="PSUM", bufs=1,
                                         name=f"aggp{h}")
                aggp_all.append(aggp)
                for j in range(NJ):
                    eng = [nc.sync, nc.scalar, nc.gpsimd][j % 3]
                    ct = wp.tile([128, GK, cw], fp8, tag=f"ct{ci}",
                                 bufs=(2 if ci == 0 else 3))
                    eng.dma_start(ct[:], C_ds[ci][j, :, :, :])
                    for i in range(GK):
                        k = j * GK + i
                        for h in range(2):
                            nc.tensor.matmul(
                                aggp[h][:, 0:cw],
                                lhsT=hbuf[:, k, h * 128:(h + 1) * 128],
                                rhs=ct[:, i, :],
                                start=(k == 0), stop=(k == NKT - 1))

            for ci, (c0, cw) in enumerate(CHUNKS):
                for h in range(2):
                    nc.vector.tensor_copy(aggT_sb[h][:, c0:c0 + cw],
                                          aggp_all[ci][h][:, 0:cw])
            # Re-materialize the gate weights with a data dependency on the
            # LAST agg chunk (zcol), so the scheduler cannot hoist phase C's
            # latency-bound chain into the agg PE stream (in-order queue
            # poison): every phase-C matmul chains off these tiles.
            zcol = wp.tile([128, 1], f32, tag="zcol")
            nc.vector.tensor_scalar_mul(zcol[:], aggp_all[2][0][:, 0:1], 0.0)
            W_ih2 = {}
            for h in range(2):
                nc.vector.tensor_scalar_add(W_ihT_sb[h][:], W_ihT_sb[h][:],
                                            zcol[:])
                t = ap_pool.tile([128, 768], f32r, tag=f"wih2{h}",
                                 name=f"wih2{h}")
                nc.vector.tensor_copy(t[:], W_ihT_sb[h][:])
                W_ih2[h] = t

            psAggB.__exit__(None, None, None)
            psAB.__exit__(None, None, None)

            # ---- phase C: LSTM (single step from zero state) + MLP ----
            with tc.tile_pool(name="psC", bufs=1, space="PSUM") as psC:
                for b in range(NBLK):
                    nn = min(128, NPC - b * 128)
                    bsl = slice(b * 128, (b + 1) * 128)
                    cbG = psC.tile([128, 768], f32, tag="cbG", space="PSUM",
                                   bufs=2)
                    cbM = psC.tile([128, 512], f32, tag="cbM", space="PSUM",
                                   bufs=2)
                    # gates: i [0:256], g [256:512], o [512:768]
                    for gi in range(3):
                        gsl = slice(gi * 256, (gi + 1) * 256)
                        for h in range(2):
                            nc.tensor.matmul(
                                cbG[:, gsl],
                                lhsT=aggT_sb[h][:, bsl],
                                rhs=W_ih2[h][:, gsl],
                                start=(h == 0), stop=(h == 1))
                    nc.vector.tensor_add(cbG[:], cbG[:], b_g_rep_sb)
                    si = wp.tile([128, 256], f32, tag="si", bufs=3)
                    nc.scalar.activation(si[:], cbG[:, 0:256], Sig,
                                         bias=0.0, scale=1.0)
                    tg = wp.tile([128, 256], f32, tag="tg", bufs=3)
                    nc.scalar.activation(tg[:], cbG[:, 256:512], Tanh,
                                         bias=0.0, scale=1.0)
                    so = wp.tile([128, 256], f32, tag="so", bufs=3)
                    nc.scalar.activation(so[:], cbG[:, 512:768], Sig,
                                         bias=0.0, scale=1.0)
                    cc = wp.tile([128, 256], f32, tag="cc", bufs=3)
                    nc.vector.tensor_mul(cc[:], si[:], tg[:])
                    tcc = wp.tile([128, 256], f32, tag="tcc", bufs=3)
                    nc.scalar.activation(tcc[:], cc[:], Tanh,
                                         bias=0.0, scale=1.0)
                    hn = wp.tile([128, 256], f32, tag="hn", bufs=3)
                    nc.vector.tensor_mul(hn[:], so[:], tcc[:])
                    hnT = {}
                    for h in range(2):
                        dst_sl = slice(h * 128, (h + 1) * 128)
                        nc.tensor.transpose(cbM[:, dst_sl],
                                            hn[:, h * 128:(h + 1) * 128],
                                            ident[:])
                        ht = wp.tile([128, 128], bf16, tag=f"hnT{h}")
                        nc.vector.tensor_copy(ht[:], cbM[:, dst_sl])
                        hnT[h] = ht
                    # MLP (transposed activation layout: [feature, node]);
                    # x1 -> cbM[256:384], x2 -> cbM[384:512], out -> cbG[0:256]
                    for h in range(2):
                        nc.tensor.matmul(cbM[:, 256:384], lhsT=W1T_sb[h],
                                         rhs=hnT[h][:],
                                         start=(h == 0), stop=(h == 1))
                    x1s = wp.tile([128, 128], bf16, tag="x1s")
                    nc.scalar.activation(x1s[:], cbM[:, 256:384], Relu,
                                         bias=b1_col_sb, scale=1.0)
                    nc.tensor.matmul(cbM[:, 384:512], lhsT=W2T_sb,
                                     rhs=x1s[:], start=True, stop=True)
                    x2s = wp.tile([128, 128], bf16, tag="x2s")
                    nc.scalar.activation(x2s[:], cbM[:, 384:512], Relu,
                                         bias=b2_col_sb, scale=1.0)
                    for oh in range(2):
                        nc.tensor.matmul(cbG[:, oh * 128:(oh + 1) * 128],
                                         lhsT=W3T_sb[:, oh * 128:(oh + 1) * 128],
                                         rhs=x2s[:], start=True, stop=True)
                        osb = wp.tile([128, 128], f32, tag=f"osb{oh}")
                        nc.vector.tensor_scalar_add(
                            osb[:], cbG[:, oh * 128:(oh + 1) * 128],
                            b3_col_sb[oh])
                        nc.gpsimd.dma_start(
                            outT_d[oh * 128:(oh + 1) * 128,
                                   b * 128:b * 128 + nn],
                            osb[:, 0:nn])

    nc.compile()
    return nc


_CACHE = {}


def _get_nc():
    if "nc" not in _CACHE:
        _CACHE["nc"] = _build()
    return _CACHE["nc"]


def prepare(inputs):
    """Build (nc, in_maps) for the SPMD run."""
    src = np.asarray(inputs["src"], dtype=np.int32)
    dst = np.asarray(inputs["dst"], dtype=np.int32)
    rel = np.asarray(inputs["rel"], dtype=np.int32)
    com = _prep_weights(inputs)
    Cs = _prep_counts(src, dst, rel)
    nc = _get_nc()
    in_maps = []
    for c in range(NCORES):
        m = dict(com)
        m.update(Cs[c])
        in_maps.append(m)
    return nc, in_maps


# ----------------------------------------------------------------------------
# public entry
# ----------------------------------------------------------------------------

def kernel(**inputs) -> np.ndarray:
    nc, in_maps = prepare(inputs)
    res = bass_utils.run_bass_kernel_spmd(nc, in_maps,
                                          core_ids=list(range(NCORES)))
    out = np.empty((N_NODES, D_OUT), dtype=np.float32)
    for c in range(NCORES):
        out[c * NPC:(c + 1) * NPC, :] = res.results[c]["outT"].T
    return out
